# revision 12
# baseline (speedup 1.0000x reference)
"""DiT block (LN -> 16-head attention -> residual -> LN -> SiLU MLP -> residual)
on 8 trn2 NeuronCores.

Sharding: core c handles batch b=c//2, sequence half s=c%2 (1024 query tokens),
with K/V computed over the full 2048-token sequence of its batch (duplicated
across the 2 cores sharing a batch; zero cross-core communication).

Layout: all activations feature-major [dim(partitions), tokens(free)] so every
linear layer is lhsT=weight[K,M] (natural layout), rhs=activation. V is built
token-major so attention-value matmuls need no transposes. Host pre-transposes
x; LN gamma/beta AND the LN mean-subtraction are folded into the following
weight matrices (centered columns: Wc = (I - 11^T/D) W, exact algebra), so the
on-device normalize is a single per-token scale by rsqrt(var+eps). Softmax runs
without max-subtraction (scores ~N(0,1)); the denominator comes from M=1
ones-matmuls col-tiled into the idle PE column-groups of the M=64 AV matmuls
(3 array-slots per 2 key-subtiles instead of 4). Stats matmuls (s1=1^Tx,
s2=1^Tx^2) are col-tiled 2-way into one PSUM bank. The tail (proj+LN2+MLP) of
sequence-tile 0 is interleaved as PE filler into the ACT-bound softmax phase of
sequence-tile 1.
"""

import sys

for _p in ("/opt/trn_rl_repo", "/root/.axon_site/_ro/trn_rl_repo"):
    if _p not in sys.path:
        sys.path.append(_p)

import numpy as np
import ml_dtypes

import concourse.bass as bass
import concourse.mybir as mybir
import concourse.tile as tile
from concourse import bacc
from concourse.bass_utils import run_bass_kernel_spmd

P = 128
D = 1024          # model dim
T = 2048          # full sequence per batch
TOWN = 1024       # query tokens per core
H = 16
HD = 64
HID = 4096
DC = D // P       # 8 dim chunks
TT = 512          # token tile
NT_FULL = T // TT     # 4
NT_OWN = TOWN // TT   # 2
HC = HID // P     # 32 hidden chunks
HG = 4            # fc hidden chunks per group
NG = HC // HG     # 4 groups
EPS = 1e-6
SCALE = HD ** -0.5

BF = mybir.dt.bfloat16
F32 = mybir.dt.float32
AF = mybir.ActivationFunctionType
ALU = mybir.AluOpType
BF_NP = ml_dtypes.bfloat16


def build_nc(loop_n=None, skip_attn=False, skip_mlp=False):
    """loop_n: if set, wrap the whole body in a hardware For_i loop that
    executes it loop_n times — used only for wall-clock-delta timing."""
    nc = bacc.Bacc(trn_type="TRN2")

    xb = nc.dram_tensor("xb", [DC, P, T], BF, kind="ExternalInput")
    xr = nc.dram_tensor("xr", [DC, P, TOWN], F32, kind="ExternalInput")
    wq = nc.dram_tensor("wq", [DC, P, DC, P], BF, kind="ExternalInput")
    wk = nc.dram_tensor("wk", [DC, P, DC, P], BF, kind="ExternalInput")
    wv = nc.dram_tensor("wv", [2, P, DC, 512], BF, kind="ExternalInput")
    wp = nc.dram_tensor("wp", [DC, P, DC, P], BF, kind="ExternalInput")
    w1 = nc.dram_tensor("w1", [HC, P, DC, P], BF, kind="ExternalInput")
    w2 = nc.dram_tensor("w2", [DC, P, NG, HG, P], BF, kind="ExternalInput")
    yo = nc.dram_tensor("y", [DC, P, TOWN], F32, kind="ExternalOutput")

    with tile.TileContext(nc) as tc:
        from contextlib import ExitStack
        with ExitStack() as loopctx:
            if loop_n is not None:
                loopctx.enter_context(tc.For_i(0, loop_n, 1))
            _body(nc, tc, xb, xr, wq, wk, wv, wp, w1, w2, yo,
                  skip_attn=skip_attn, skip_mlp=skip_mlp)

    nc.finalize()
    return nc


def _body(nc, tc, xb, xr, wq, wk, wv, wp, w1, w2, yo,
          skip_attn=False, skip_mlp=False):
    xb_pm = xb[:].rearrange("c p t -> p c t")
    xr_pm = xr[:].rearrange("c p t -> p c t")
    from contextlib import ExitStack
    with ExitStack() as ctx:
        pers = ctx.enter_context(tc.tile_pool(name="pers", bufs=1))
        pp2 = ctx.enter_context(tc.tile_pool(name="pp2", bufs=2, space="PSUM"))
        pp1 = ctx.enter_context(tc.tile_pool(name="pp1", bufs=1, space="PSUM"))

        # persistent activations
        xhat = pers.tile([P, DC, T], BF, tag="xhat")
        q_sb = pers.tile([P, DC, TOWN], BF, tag="q_sb")
        k_sb = pers.tile([P, DC, T], BF, tag="k_sb")
        v_sb = pers.tile([P, T // P, H, HD], BF, tag="v_sb")
        attn = pers.tile([P, DC, TOWN], BF, tag="attn")
        consts = pers.tile([P, 2], F32, tag="consts")
        eps_t = consts[:, 0:1]
        ones_bf = consts[:, 1:2].bitcast(BF)[:, 0:1]

        nc.vector.memset(eps_t, EPS)
        nc.vector.memset(ones_bf, 1.0)

        def stats_pair(src_getter, out_row):
            """2-way col-tiled stats chains: s1=1^T x at psum partition 0,
            s2=1^T x^2 at partition 32 of the same bank (concurrent on PE)."""
            st = pp1.tile([64, TT], F32, tag="ta")
            for c in range(DC):
                src, sq = src_getter(c)
                nc.tensor.matmul(st[0:1, :], lhsT=ones_bf, rhs=src,
                                 start=(c == 0), stop=(c == DC - 1))
                nc.tensor.matmul(st[32:33, :], lhsT=ones_bf, rhs=sq,
                                 start=(c == 0), stop=(c == DC - 1))
            return st

        def finish_stats(st, bbpool, rowpool, spool):
            """[1,TT] math: rs = 1/sqrt(s2/D - (s1/D)^2 + eps); broadcast."""
            mu = spool.tile([1, TT], F32, tag="sq")
            rs = rowpool.tile([1, TT], F32, tag="strow")
            mu, rs = mu[:], rs[:]
            nc.vector.tensor_scalar_mul(mu, st[0:1, :], 1.0 / D)
            nc.vector.tensor_mul(rs, mu, mu)
            nc.vector.scalar_tensor_tensor(rs, st[32:33, :], 1.0 / D, rs,
                                           op0=ALU.mult, op1=ALU.subtract)
            nc.scalar.activation(rs, rs, AF.Sqrt, bias=eps_t[0:1, :])
            nc.vector.reciprocal(rs, rs)
            bb = bbpool.tile([P, TT], F32, tag="bb")
            nc.gpsimd.partition_broadcast(bb[:], rs, channels=P)
            return bb

        # ------- region 1: LN1 + V, then K/Q
        with tc.tile_pool(name="ph_ab", bufs=2) as spool, \
             tc.tile_pool(name="xload", bufs=1) as xpool, \
             tc.tile_pool(name="wvp", bufs=1) as wvpool, \
             tc.tile_pool(name="wkqp", bufs=1) as wpool, \
             tc.tile_pool(name="pp_c", bufs=2) as ppool, \
             tc.tile_pool(name="ph_c", bufs=1) as cpool, \
             tc.tile_pool(name="ph_f", bufs=2) as fpool, \
             tc.tile_pool(name="w2p", bufs=2) as w2pool, \
             tc.tile_pool(name="xrp", bufs=1) as xrpool, \
             tc.tile_pool(name="ph_e", bufs=1) as gpool:

            def emit_v(og, mt_range, wvt):
                for mt in mt_range:
                    ps = pp1.tile([P, 512], F32, tag="av" if mt % 2 == 0 else "den")
                    for c in range(DC):
                        nc.tensor.matmul(ps[:],
                                         lhsT=xhat[:, c, mt * P:(mt + 1) * P],
                                         rhs=wvt[:, c, :],
                                         start=(c == 0), stop=(c == DC - 1))
                    nc.vector.tensor_copy(
                        v_sb[:, mt, og * 8:(og + 1) * 8, :],
                        ps[:].rearrange("p (h d) -> p h d", h=8))

            wv0 = wvpool.tile([P, DC, 512], BF, tag="wv")
            nc.sync.dma_start(wv0[:], wv[0, :, :, :])
            for nt in range(NT_FULL):
                xt = xpool.tile([P, DC, TT], BF, tag="xt")
                nc.sync.dma_start(xt[:], xb_pm[:, :, nt * TT:(nt + 1) * TT])

                def src_sq(c, _xt=xt):
                    src = _xt[:, c, :]
                    sq = spool.tile([P, TT], BF, tag="sq")
                    nc.vector.tensor_mul(sq[:], src, src)
                    return src, sq[:]
                st = stats_pair(src_sq, 0)
                bb = finish_stats(st, gpool, cpool, spool)
                for c in range(DC):
                    nc.vector.tensor_mul(xhat[:, c, nt * TT:(nt + 1) * TT],
                                         xt[:, c, :], bb)
                emit_v(0, range(nt * (TT // P), (nt + 1) * (TT // P)), wv0)
            wv1 = wvpool.tile([P, DC, 512], BF, tag="wv")
            nc.sync.dma_start(wv1[:], wv[1, :, :, :])
            emit_v(1, range(T // P), wv1)

            def emit_kq(hp):
                """Generator: yields after each psum accumulation group."""
                wkt = wpool.tile([P, DC, P], BF, tag="wkq")
                nc.sync.dma_start(wkt[:], wk[hp, :, :, :])
                for nt in range(NT_FULL):
                    sl = slice(nt * TT, (nt + 1) * TT)
                    ps = pp1.tile([P, TT], F32, tag="ta" if nt % 2 == 0 else "tb")
                    for c in range(DC):
                        nc.tensor.matmul(ps[:], lhsT=wkt[:, c, :], rhs=xhat[:, c, sl],
                                         start=(c == 0), stop=(c == DC - 1))
                    nc.vector.tensor_copy(k_sb[:, hp, sl], ps[:])
                    yield
                wqt = wpool.tile([P, DC, P], BF, tag="wkq")
                nc.sync.dma_start(wqt[:], wq[hp, :, :, :])
                for nt in range(NT_OWN):
                    sl = slice(nt * TT, (nt + 1) * TT)
                    ps = pp1.tile([P, TT], F32, tag="ta" if nt % 2 == 0 else "tb")
                    for c in range(DC):
                        nc.tensor.matmul(ps[:], lhsT=wqt[:, c, :], rhs=xhat[:, c, sl],
                                         start=(c == 0), stop=(c == DC - 1))
                    nc.vector.tensor_copy(q_sb[:, hp, sl], ps[:])
                    yield

            def emit_attn(hp, n):
                """Generator: yields after each j step. AV matmuls are M=64
                col-tiled pairs (head A cols 0:64 -> psum rows 0:64, head B
                cols 64:128 -> rows 64:128 of one bank); softmax-denominator
                ones-matmuls ride the leftover col groups (3 slots per 2
                key-subtiles). Den rows: 64=A-even, 96=B-even, 0=A-odd,
                32=B-odd of a second bank."""
                nsl = slice(n * TT, (n + 1) * TT)
                av = pp1.tile([P, TT], F32, tag="av")
                den = pp1.tile([P, TT], F32, tag="den")
                NJ = T // P // 2

                def av_slots(j, pt):
                    m0, m1 = 2 * j, 2 * j + 1
                    vA0 = v_sb[:, m0, 2 * hp, :]
                    vB0 = v_sb[:, m0, 2 * hp + 1, :]
                    vA1 = v_sb[:, m1, 2 * hp, :]
                    vB1 = v_sb[:, m1, 2 * hp + 1, :]
                    # slot 1: A(m0) cols 0:64  ||  B(m0) cols 64:128
                    nc.tensor.matmul(av[0:HD, :], lhsT=vA0, rhs=pt[:, 0, :],
                                     start=(m0 == 0), stop=False)
                    nc.tensor.matmul(av[HD:P, :], lhsT=vB0, rhs=pt[:, 2, :],
                                     start=(m0 == 0), stop=False)
                    # slot 2: A(m1) cols 0:64 || denA(m0)@64 || denB(m0)@96
                    nc.tensor.matmul(av[0:HD, :], lhsT=vA1, rhs=pt[:, 1, :],
                                     start=False, stop=(m1 == T // P - 1))
                    nc.tensor.matmul(den[64:65, :], lhsT=ones_bf, rhs=pt[:, 0, :],
                                     start=(j == 0), stop=(j == NJ - 1))
                    nc.tensor.matmul(den[96:97, :], lhsT=ones_bf, rhs=pt[:, 2, :],
                                     start=(j == 0), stop=(j == NJ - 1),
                                     tile_position=(0, 96))
                    # slot 3: B(m1) cols 64:128 || denA(m1)@0 || denB(m1)@32
                    nc.tensor.matmul(av[HD:P, :], lhsT=vB1, rhs=pt[:, 3, :],
                                     start=False, stop=(m1 == T // P - 1))
                    nc.tensor.matmul(den[0:1, :], lhsT=ones_bf, rhs=pt[:, 1, :],
                                     start=(j == 0), stop=(j == NJ - 1))
                    nc.tensor.matmul(den[32:33, :], lhsT=ones_bf, rhs=pt[:, 3, :],
                                     start=(j == 0), stop=(j == NJ - 1))

                prev = None
                for j in range(NJ):
                    scA = pp2.tile([P, 2, TT], F32, tag="sc")
                    scB = pp2.tile([P, 2, TT], F32, tag="sc")
                    for jj in range(2):
                        m = 2 * j + jj
                        msl = slice(m * P, (m + 1) * P)
                        nc.tensor.matmul(scA[:, jj, :],
                                         lhsT=k_sb[0:HD, hp, msl],
                                         rhs=q_sb[0:HD, hp, nsl],
                                         start=True, stop=True)
                        nc.tensor.matmul(scB[:, jj, :],
                                         lhsT=k_sb[HD:P, hp, msl],
                                         rhs=q_sb[HD:P, hp, nsl],
                                         start=True, stop=True)
                    pt = ppool.tile([P, 4, TT], BF, tag="p")
                    nc.scalar.activation(pt[:, 0:2, :], scA[:], AF.Exp, scale=SCALE)
                    nc.scalar.activation(pt[:, 2:4, :], scB[:], AF.Exp, scale=SCALE)
                    if prev is not None:
                        av_slots(j - 1, prev)
                    prev = pt
                    yield
                av_slots(NJ - 1, prev)

                # divide by summed-exp. Evict PSUM->SBUF immediately so the
                # av/den banks free fast; the divide chain runs off SBUF.
                avs = cpool.tile([P, TT], F32, tag="avs")
                nc.vector.tensor_copy(avs[:], av[:])
                dsb = cpool.tile([P, TT], F32, tag="dsb")
                nc.vector.tensor_copy(dsb[:], den[:])
                # align the four den rows for lane-locked DVE adds
                sh = cpool.tile([1, 3, TT], F32, tag="sh")
                nc.sync.dma_start(sh[:, 0, :], dsb[64:65, :])   # A even
                nc.sync.dma_start(sh[:, 1, :], dsb[32:33, :])   # B odd
                nc.sync.dma_start(sh[:, 2, :], dsb[96:97, :])   # B even
                nc.vector.tensor_add(sh[:, 0, :], dsb[0:1, :], sh[:, 0, :])
                nc.vector.tensor_add(sh[:, 1, :], sh[:, 1, :], sh[:, 2, :])
                nc.vector.reciprocal(sh[:, 0, :], sh[:, 0, :])
                nc.vector.reciprocal(sh[:, 1, :], sh[:, 1, :])
                rbA = cpool.tile([HD, TT], F32, tag="rbA")
                nc.gpsimd.partition_broadcast(rbA[:], sh[:, 0, :], channels=HD)
                rbB = cpool.tile([P, TT], F32, tag="rbB")
                nc.gpsimd.partition_broadcast(rbB[:], sh[:, 1, :], channels=P)
                nc.vector.tensor_mul(attn[0:HD, hp, nsl], avs[0:HD, :], rbA[:])
                nc.vector.tensor_mul(attn[HD:P, hp, nsl], avs[HD:P, :],
                                     rbB[HD:P, :])

            # ------- tail(n): proj + LN2 + grouped MLP (generator for filler)
            # reuse the 32KB xhat slot for proj weights (16KB) + xh2 (8KB)
            wx = pers.tile([P, DC, DC * P + TT], BF, tag="xhat")
            wpt = wx[:, :, 0:DC * P].rearrange("p o (c m) -> p o c m", c=DC)
            xh2 = wx[:, :, DC * P:DC * P + TT]
            h_sb = pers.tile([P, DC, TT], F32, tag="h")

            def tail(n):
                nsl = slice(n * TT, (n + 1) * TT)
                for o in range(DC):
                    xo = xrpool.tile([P, TT], F32, tag="xo")
                    nc.sync.dma_start(xo[:], xr_pm[:, o, nsl])
                    ps = pp1.tile([P, TT], F32, tag="ta" if o % 2 == 0 else "tb")
                    for c in range(DC):
                        nc.tensor.matmul(ps[:], lhsT=wpt[:, o, c, :],
                                         rhs=attn[:, c, nsl],
                                         start=(c == 0), stop=(c == DC - 1))
                    nc.vector.tensor_add(h_sb[:, o, :], ps[:], xo[:])
                    yield
                if skip_mlp:
                    for o in range(DC):
                        nc.sync.dma_start(
                            yo[:].rearrange("c p t -> p c t")[:, o, nsl],
                            h_sb[:, o, :])
                        yield
                    return
                # LN2 (stats on bf16 copies; normalize = scale-only, folded
                # mean via centered w1)
                def src_sq2(c):
                    hb = spool.tile([P, TT], BF, tag="hb")
                    nc.vector.tensor_copy(hb[:], h_sb[:, c, :])
                    sq = spool.tile([P, TT], BF, tag="sq")
                    nc.vector.tensor_mul(sq[:], hb[:], hb[:])
                    return hb[:], sq[:]
                st = stats_pair(src_sq2, 0)
                yield
                bb = finish_stats(st, gpool, cpool, spool)
                for c in range(DC):
                    nc.vector.tensor_mul(xh2[:, c, :], h_sb[:, c, :], bb)
                yield
                # grouped MLP: fc1/silu for 8 hidden chunks, then partial fc2
                # accumulated into h_sb (residual base) in-place
                for grp in range(NG):
                    g = gpool.tile([P, HG, TT], BF, tag="g")
                    for k in range(HG):
                        hc = grp * HG + k
                        w1t = fpool.tile([P, DC, P], BF, tag="w1")
                        nc.sync.dma_start(w1t[:], w1[hc, :, :, :])
                        ps = pp1.tile([P, TT], F32, tag="ta" if k % 2 == 0 else "tb")
                        for c in range(DC):
                            nc.tensor.matmul(ps[:], lhsT=w1t[:, c, :],
                                             rhs=xh2[:, c, :],
                                             start=(c == 0), stop=(c == DC - 1))
                        nc.scalar.activation(g[:, k, :], ps[:], AF.Silu)
                        yield
                    for o in range(DC):
                        w2t = w2pool.tile([P, HG, P], BF, tag="w2")
                        nc.sync.dma_start(w2t[:], w2[o, :, grp, :, :])
                        ps = pp1.tile([P, TT], F32, tag="ta" if o % 2 == 0 else "tb")
                        for k in range(HG):
                            nc.tensor.matmul(ps[:], lhsT=w2t[:, k, :],
                                             rhs=g[:, k, :],
                                             start=(k == 0), stop=(k == HG - 1))
                        nc.vector.tensor_add(h_sb[:, o, :], h_sb[:, o, :], ps[:])
                        yield
                for o in range(DC):
                    nc.sync.dma_start(
                        yo[:].rearrange("c p t -> p c t")[:, o, nsl],
                        h_sb[:, o, :])
                yield

            def drain(gen):
                if gen is not None:
                    for _ in gen:
                        pass

            nc.sync.dma_start(wpt, wp[:].rearrange("o p c m -> p o c m"))

            if skip_attn:
                nc.vector.memset(attn[:], 0.01)
                for hp in range(DC):
                    drain(emit_kq(hp))
                drain(tail(0))
                drain(tail(1))
                return

            # n=0: interleave attention with K/Q of the next head-pair.
            # n=1: interleave attention with tail(0) so PE chews proj/MLP of
            # sequence-half 0 while ACT runs the exp wave of half 1.
            drain(emit_kq(0))
            for hp in range(DC):
                kq_next = emit_kq(hp + 1) if hp + 1 < DC else None
                for step, _ in enumerate(emit_attn(hp, 0)):
                    if kq_next is not None and step % 2 == 1:
                        next(kq_next, None)
                        next(kq_next, None)
                drain(kq_next)
            tail0 = tail(0)
            for hp in range(DC):
                for _ in emit_attn(hp, 1):
                    next(tail0, None)
            drain(tail0)
            drain(tail(1))


_CACHE = {}


def _get_nc():
    if "nc" not in _CACHE:
        _CACHE["nc"] = build_nc()
    return _CACHE["nc"]


def _prep_weights(ln1_w, ln1_b, qkv_w, qkv_b, proj_w, proj_b,
                  ln2_w, ln2_b, fc1_w, fc1_b, fc2_w, fc2_b):
    """Fold LN affine params AND the LN mean-subtraction into the adjacent
    weights (column centering; exact algebra), lay out as [K-chunk, K-in-chunk,
    M] bf16."""
    qkv_w = np.asarray(qkv_w, np.float32)
    fold1 = np.asarray(ln1_w, np.float32)[:, None] * qkv_w
    bias1 = np.asarray(qkv_b, np.float32) + np.asarray(ln1_b, np.float32) @ qkv_w
    fold1 = fold1 - fold1.mean(axis=0, keepdims=True)   # center: folds mean-sub
    fc1 = np.asarray(fc1_w, np.float32)
    fold2 = np.asarray(ln2_w, np.float32)[:, None] * fc1
    bias2 = np.asarray(fc1_b, np.float32) + np.asarray(ln2_b, np.float32) @ fc1
    fold2 = fold2 - fold2.mean(axis=0, keepdims=True)
    for name, b in (("qkv", bias1), ("proj", np.asarray(proj_b, np.float32)),
                    ("fc1", bias2), ("fc2", np.asarray(fc2_b, np.float32))):
        assert not np.any(b), (
            f"{name} effective bias is nonzero; bias support not emitted in this kernel")

    def chunk4(a, n_out, m_out):
        # [D_in, D_out] -> [out-chunk, p, in-chunk, m] with contiguous per-p tile
        return np.ascontiguousarray(
            a.reshape(a.shape[0] // P, P, n_out, m_out).transpose(2, 1, 0, 3)
        ).astype(BF_NP)

    wq_ = chunk4(fold1[:, 0:D], DC, P)
    wk_ = chunk4(fold1[:, D:2 * D], DC, P)
    wv_ = chunk4(fold1[:, 2 * D:3 * D], 2, 512)
    wp_ = chunk4(np.asarray(proj_w, np.float32), DC, P)
    w1_ = chunk4(fold2, HC, P)
    w2_ = chunk4(np.asarray(fc2_w, np.float32), DC, P)
    # fc2 grouped layout: [DC, P, HC, P] -> [DC, P, NG, HG, P]
    w2_ = np.ascontiguousarray(w2_.reshape(DC, P, NG, HG, P))
    return dict(wq=wq_, wk=wk_, wv=wv_, wp=wp_, w1=w1_, w2=w2_)


def kernel(x, ln1_w, ln1_b, qkv_w, qkv_b, proj_w, proj_b,
           ln2_w, ln2_b, fc1_w, fc1_b, fc2_w, fc2_b):
    x = np.asarray(x, np.float32)
    B = x.shape[0]
    assert x.shape == (B, T, D) and B * 2 == 8, f"unexpected x shape {x.shape}"
    weights = _prep_weights(ln1_w, ln1_b, qkv_w, qkv_b, proj_w, proj_b,
                            ln2_w, ln2_b, fc1_w, fc1_b, fc2_w, fc2_b)
    nc = _get_nc()

    in_maps = []
    for c in range(8):
        b, s = c // 2, c % 2
        if s == 0:
            xp = x[b]
        else:
            xp = np.concatenate([x[b, TOWN:], x[b, :TOWN]], axis=0)
        xb_ = np.ascontiguousarray(xp.T).reshape(DC, P, T).astype(BF_NP)
        xr_ = np.ascontiguousarray(xp[:TOWN].T).reshape(DC, P, TOWN)
        in_maps.append({"xb": xb_, "xr": xr_, **weights})

    res = run_bass_kernel_spmd(nc, in_maps, core_ids=list(range(8)))

    y = np.empty((B, T, D), np.float32)
    for c in range(8):
        b, s = c // 2, c % 2
        yc = res.results[c]["y"].reshape(D, TOWN)  # [dim, own tokens]
        y[b, s * TOWN:(s + 1) * TOWN, :] = yc.T
    return y


# revision 13
# speedup vs baseline: 1.1564x; 1.1564x over previous
"""DiT block (LN -> 16-head attention -> residual -> LN -> SiLU MLP -> residual)
on 8 trn2 NeuronCores.

Sharding: core c handles batch b=c//2, sequence half s=c%2 (1024 query tokens),
with K/V computed over the full 2048-token sequence of its batch (duplicated
across the 2 cores sharing a batch; zero cross-core communication).

Layout: all activations feature-major [dim(partitions), tokens(free)] so every
linear layer is lhsT=weight[K,M] (natural layout), rhs=activation. V is built
token-major so attention-value matmuls need no transposes. Host pre-transposes
x; LN gamma/beta AND the LN mean-subtraction are folded into the following
weight matrices (centered columns: Wc = (I - 11^T/D) W, exact algebra), so the
on-device normalize is a single per-token scale by rsqrt(var+eps). Softmax runs
without max-subtraction (scores ~N(0,1)); the denominator comes from M=1
ones-matmuls col-tiled into the idle PE column-groups of the M=64 AV matmuls
(3 array-slots per 2 key-subtiles instead of 4). Stats matmuls (s1=1^Tx,
s2=1^Tx^2) are col-tiled 2-way into one PSUM bank. The tail (proj+LN2+MLP) of
sequence-tile 0 is interleaved as PE filler into the ACT-bound softmax phase of
sequence-tile 1.
"""

import sys

for _p in ("/opt/trn_rl_repo", "/root/.axon_site/_ro/trn_rl_repo"):
    if _p not in sys.path:
        sys.path.append(_p)

import numpy as np
import ml_dtypes

import concourse.bass as bass
import concourse.mybir as mybir
import concourse.tile as tile
from concourse import bacc
from concourse.bass_utils import run_bass_kernel_spmd

P = 128
D = 1024          # model dim
T = 2048          # full sequence per batch
TOWN = 1024       # query tokens per core
H = 16
HD = 64
HID = 4096
DC = D // P       # 8 dim chunks
TT = 512          # token tile
NT_FULL = T // TT     # 4
NT_OWN = TOWN // TT   # 2
HC = HID // P     # 32 hidden chunks
HG = 4            # fc hidden chunks per group
NG = HC // HG     # 4 groups
EPS = 1e-6
SCALE = HD ** -0.5

BF = mybir.dt.bfloat16
F32 = mybir.dt.float32
AF = mybir.ActivationFunctionType
ALU = mybir.AluOpType
BF_NP = ml_dtypes.bfloat16


def build_nc(loop_n=None, skip_attn=False, skip_mlp=False):
    """loop_n: if set, wrap the whole body in a hardware For_i loop that
    executes it loop_n times — used only for wall-clock-delta timing."""
    nc = bacc.Bacc(trn_type="TRN2")

    xb = nc.dram_tensor("xb", [DC, P, T], BF, kind="ExternalInput")
    xr = nc.dram_tensor("xr", [DC, P, TOWN], F32, kind="ExternalInput")
    wq = nc.dram_tensor("wq", [DC, P, DC, P], BF, kind="ExternalInput")
    wk = nc.dram_tensor("wk", [DC, P, DC, P], BF, kind="ExternalInput")
    wv = nc.dram_tensor("wv", [2, P, DC, 512], BF, kind="ExternalInput")
    wp = nc.dram_tensor("wp", [DC, P, DC, P], BF, kind="ExternalInput")
    w1 = nc.dram_tensor("w1", [HC, P, DC, P], BF, kind="ExternalInput")
    w2 = nc.dram_tensor("w2", [DC, P, NG, HG, P], BF, kind="ExternalInput")
    yo = nc.dram_tensor("y", [DC, P, TOWN], F32, kind="ExternalOutput")

    with tile.TileContext(nc) as tc:
        from contextlib import ExitStack
        with ExitStack() as loopctx:
            if loop_n is not None:
                loopctx.enter_context(tc.For_i(0, loop_n, 1))
            _body(nc, tc, xb, xr, wq, wk, wv, wp, w1, w2, yo,
                  skip_attn=skip_attn, skip_mlp=skip_mlp)

    nc.finalize()
    return nc


def _body(nc, tc, xb, xr, wq, wk, wv, wp, w1, w2, yo,
          skip_attn=False, skip_mlp=False):
    xb_pm = xb[:].rearrange("c p t -> p c t")
    xr_pm = xr[:].rearrange("c p t -> p c t")
    from contextlib import ExitStack
    with ExitStack() as ctx:
        pers = ctx.enter_context(tc.tile_pool(name="pers", bufs=1))
        pp2 = ctx.enter_context(tc.tile_pool(name="pp2", bufs=2, space="PSUM"))
        pp1 = ctx.enter_context(tc.tile_pool(name="pp1", bufs=1, space="PSUM"))

        # persistent activations
        xhat = pers.tile([P, DC, T], BF, tag="xhat")
        q_sb = pers.tile([P, DC, TOWN], BF, tag="q_sb")
        k_sb = pers.tile([P, DC, T], BF, tag="k_sb")
        v_sb = pers.tile([P, T // P, H, HD], BF, tag="v_sb")
        attn = pers.tile([P, DC, TOWN], BF, tag="attn")
        consts = pers.tile([P, 2], F32, tag="consts")
        eps_t = consts[:, 0:1]
        ones_bf = consts[:, 1:2].bitcast(BF)[:, 0:1]

        nc.vector.memset(eps_t, EPS)
        nc.vector.memset(ones_bf, 1.0)

        def stats_pair(src_getter, out_row):
            """2-way col-tiled stats chains: s1=1^T x at psum partition 0,
            s2=1^T x^2 at partition 32 of the same bank (concurrent on PE)."""
            st = pp1.tile([64, TT], F32, tag="ta")
            for c in range(DC):
                src, sq = src_getter(c)
                nc.tensor.matmul(st[0:1, :], lhsT=ones_bf, rhs=src,
                                 start=(c == 0), stop=(c == DC - 1))
                nc.tensor.matmul(st[32:33, :], lhsT=ones_bf, rhs=sq,
                                 start=(c == 0), stop=(c == DC - 1))
            return st

        def finish_stats(st, bbpool, rowpool, spool):
            """[1,TT] math: rs = 1/sqrt(s2/D - (s1/D)^2 + eps); broadcast."""
            mu = spool.tile([1, TT], F32, tag="sq")
            rs = rowpool.tile([1, TT], F32, tag="strow")
            mu, rs = mu[:], rs[:]
            nc.vector.tensor_scalar_mul(mu, st[0:1, :], 1.0 / D)
            nc.vector.tensor_mul(rs, mu, mu)
            nc.vector.scalar_tensor_tensor(rs, st[32:33, :], 1.0 / D, rs,
                                           op0=ALU.mult, op1=ALU.subtract)
            nc.scalar.activation(rs, rs, AF.Sqrt, bias=eps_t[0:1, :])
            nc.vector.reciprocal(rs, rs)
            bb = bbpool.tile([P, TT], F32, tag="bb")
            nc.gpsimd.partition_broadcast(bb[:], rs, channels=P)
            return bb

        # ------- region 1: LN1 + V, then K/Q
        with tc.tile_pool(name="ph_ab", bufs=2) as spool, \
             tc.tile_pool(name="wvp", bufs=1) as wvpool, \
             tc.tile_pool(name="wkqp", bufs=2) as wpool, \
             tc.tile_pool(name="pp_c", bufs=3) as ppool, \
             tc.tile_pool(name="ph_c", bufs=1) as cpool, \
             tc.tile_pool(name="ph_f", bufs=2) as fpool, \
             tc.tile_pool(name="w2p", bufs=2) as w2pool, \
             tc.tile_pool(name="xrp", bufs=2) as xrpool, \
             tc.tile_pool(name="ph_e", bufs=1) as gpool:

            def emit_v(og, mt_range, wvt):
                for mt in mt_range:
                    ps = pp1.tile([P, 512], F32, tag="av" if mt % 2 == 0 else "den")
                    for c in range(DC):
                        nc.tensor.matmul(ps[:],
                                         lhsT=xhat[:, c, mt * P:(mt + 1) * P],
                                         rhs=wvt[:, c, :],
                                         start=(c == 0), stop=(c == DC - 1))
                    nc.vector.tensor_copy(
                        v_sb[:, mt, og * 8:(og + 1) * 8, :],
                        ps[:].rearrange("p (h d) -> p h d", h=8))

            wv0 = wvpool.tile([P, DC, 512], BF, tag="wv")
            nc.sync.dma_start(wv0[:], wv[0, :, :, :])
            for nt in range(NT_FULL):
                nsl = slice(nt * TT, (nt + 1) * TT)
                nc.sync.dma_start(xhat[:, :, nsl], xb_pm[:, :, nsl])

                def src_sq(c, _nsl=nsl):
                    src = xhat[:, c, _nsl]
                    sq = spool.tile([P, TT], BF, tag="sq")
                    nc.vector.tensor_mul(sq[:], src, src)
                    return src, sq[:]
                st = stats_pair(src_sq, 0)
                bb = finish_stats(st, gpool, cpool, spool)
                for c in range(DC):
                    nc.vector.tensor_mul(xhat[:, c, nsl], xhat[:, c, nsl], bb)
                emit_v(0, range(nt * (TT // P), (nt + 1) * (TT // P)), wv0)
            wv1 = wvpool.tile([P, DC, 512], BF, tag="wv")
            nc.sync.dma_start(wv1[:], wv[1, :, :, :])
            emit_v(1, range(T // P), wv1)

            def emit_kq(hp):
                """Generator: yields after each psum accumulation group."""
                wkt = wpool.tile([P, DC, P], BF, tag="wkq")
                nc.sync.dma_start(wkt[:], wk[hp, :, :, :])
                for nt in range(NT_FULL):
                    sl = slice(nt * TT, (nt + 1) * TT)
                    ps = pp1.tile([P, TT], F32, tag="ta" if nt % 2 == 0 else "tb")
                    for c in range(DC):
                        nc.tensor.matmul(ps[:], lhsT=wkt[:, c, :], rhs=xhat[:, c, sl],
                                         start=(c == 0), stop=(c == DC - 1))
                    nc.vector.tensor_copy(k_sb[:, hp, sl], ps[:])
                    yield
                wqt = wpool.tile([P, DC, P], BF, tag="wkq")
                nc.sync.dma_start(wqt[:], wq[hp, :, :, :])
                for nt in range(NT_OWN):
                    sl = slice(nt * TT, (nt + 1) * TT)
                    ps = pp1.tile([P, TT], F32, tag="ta" if nt % 2 == 0 else "tb")
                    for c in range(DC):
                        nc.tensor.matmul(ps[:], lhsT=wqt[:, c, :], rhs=xhat[:, c, sl],
                                         start=(c == 0), stop=(c == DC - 1))
                    nc.vector.tensor_copy(q_sb[:, hp, sl], ps[:])
                    yield

            def emit_attn(hp, n):
                """Generator: yields after each j step. AV matmuls are M=64
                col-tiled pairs (head A cols 0:64 -> psum rows 0:64, head B
                cols 64:128 -> rows 64:128 of one bank); softmax-denominator
                ones-matmuls ride the leftover col groups (3 slots per 2
                key-subtiles). Den rows: 64=A-even, 96=B-even, 0=A-odd,
                32=B-odd of a second bank."""
                nsl = slice(n * TT, (n + 1) * TT)
                av = pp1.tile([P, TT], F32, tag="av")
                den = pp1.tile([P, TT], F32, tag="den")
                NJ = T // P // 2

                def av_slots(j, pt):
                    m0, m1 = 2 * j, 2 * j + 1
                    vA0 = v_sb[:, m0, 2 * hp, :]
                    vB0 = v_sb[:, m0, 2 * hp + 1, :]
                    vA1 = v_sb[:, m1, 2 * hp, :]
                    vB1 = v_sb[:, m1, 2 * hp + 1, :]
                    # slot 1: A(m0) cols 0:64  ||  B(m0) cols 64:128
                    nc.tensor.matmul(av[0:HD, :], lhsT=vA0, rhs=pt[:, 0, :],
                                     start=(m0 == 0), stop=False)
                    nc.tensor.matmul(av[HD:P, :], lhsT=vB0, rhs=pt[:, 2, :],
                                     start=(m0 == 0), stop=False)
                    # slot 2: A(m1) cols 0:64 || denA(m0)@64 || denB(m0)@96
                    nc.tensor.matmul(av[0:HD, :], lhsT=vA1, rhs=pt[:, 1, :],
                                     start=False, stop=(m1 == T // P - 1))
                    nc.tensor.matmul(den[64:65, :], lhsT=ones_bf, rhs=pt[:, 0, :],
                                     start=(j == 0), stop=(j == NJ - 1))
                    nc.tensor.matmul(den[96:97, :], lhsT=ones_bf, rhs=pt[:, 2, :],
                                     start=(j == 0), stop=(j == NJ - 1),
                                     tile_position=(0, 96))
                    # slot 3: B(m1) cols 64:128 || denA(m1)@0 || denB(m1)@32
                    nc.tensor.matmul(av[HD:P, :], lhsT=vB1, rhs=pt[:, 3, :],
                                     start=False, stop=(m1 == T // P - 1))
                    nc.tensor.matmul(den[0:1, :], lhsT=ones_bf, rhs=pt[:, 1, :],
                                     start=(j == 0), stop=(j == NJ - 1))
                    nc.tensor.matmul(den[32:33, :], lhsT=ones_bf, rhs=pt[:, 3, :],
                                     start=(j == 0), stop=(j == NJ - 1))

                prev = None
                for j in range(NJ):
                    scA = pp2.tile([P, 2, TT], F32, tag="sc")
                    scB = pp2.tile([P, 2, TT], F32, tag="sc")
                    for jj in range(2):
                        m = 2 * j + jj
                        msl = slice(m * P, (m + 1) * P)
                        nc.tensor.matmul(scA[:, jj, :],
                                         lhsT=k_sb[0:HD, hp, msl],
                                         rhs=q_sb[0:HD, hp, nsl],
                                         start=True, stop=True)
                        nc.tensor.matmul(scB[:, jj, :],
                                         lhsT=k_sb[HD:P, hp, msl],
                                         rhs=q_sb[HD:P, hp, nsl],
                                         start=True, stop=True)
                    pt = ppool.tile([P, 4, TT], BF, tag="p")
                    nc.scalar.activation(pt[:, 0:2, :], scA[:], AF.Exp, scale=SCALE)
                    nc.scalar.activation(pt[:, 2:4, :], scB[:], AF.Exp, scale=SCALE)
                    if prev is not None:
                        av_slots(j - 1, prev)
                    prev = pt
                    yield
                av_slots(NJ - 1, prev)

                # divide by summed-exp. Evict PSUM->SBUF immediately so the
                # av/den banks free fast; the divide chain runs off SBUF.
                avs = cpool.tile([P, TT], F32, tag="avs")
                nc.vector.tensor_copy(avs[:], av[:])
                dsb = cpool.tile([P, TT], F32, tag="dsb")
                nc.vector.tensor_copy(dsb[:], den[:])
                # align the four den rows for lane-locked DVE adds
                sh = cpool.tile([1, 3, TT], F32, tag="sh")
                nc.sync.dma_start(sh[:, 0, :], dsb[64:65, :])   # A even
                nc.sync.dma_start(sh[:, 1, :], dsb[32:33, :])   # B odd
                nc.sync.dma_start(sh[:, 2, :], dsb[96:97, :])   # B even
                nc.vector.tensor_add(sh[:, 0, :], dsb[0:1, :], sh[:, 0, :])
                nc.vector.tensor_add(sh[:, 1, :], sh[:, 1, :], sh[:, 2, :])
                nc.vector.reciprocal(sh[:, 0, :], sh[:, 0, :])
                nc.vector.reciprocal(sh[:, 1, :], sh[:, 1, :])
                rbA = cpool.tile([HD, TT], F32, tag="rbA")
                nc.gpsimd.partition_broadcast(rbA[:], sh[:, 0, :], channels=HD)
                rbB = cpool.tile([P, TT], F32, tag="rbB")
                nc.gpsimd.partition_broadcast(rbB[:], sh[:, 1, :], channels=P)
                nc.vector.tensor_mul(attn[0:HD, hp, nsl], avs[0:HD, :], rbA[:])
                nc.vector.tensor_mul(attn[HD:P, hp, nsl], avs[HD:P, :],
                                     rbB[HD:P, :])

            # ------- tail(n): proj + LN2 + grouped MLP (generator for filler)
            # reuse the 32KB xhat slot for proj weights (16KB) + xh2 (8KB)
            wx = pers.tile([P, DC, DC * P + TT], BF, tag="xhat")
            wpt = wx[:, :, 0:DC * P].rearrange("p o (c m) -> p o c m", c=DC)
            xh2 = wx[:, :, DC * P:DC * P + TT]
            h_sb = pers.tile([P, DC, TT], F32, tag="h")

            def tail(n):
                nsl = slice(n * TT, (n + 1) * TT)
                for o in range(DC):
                    xo = xrpool.tile([P, TT], F32, tag="xo")
                    nc.sync.dma_start(xo[:], xr_pm[:, o, nsl])
                    ps = pp1.tile([P, TT], F32, tag="ta" if o % 2 == 0 else "tb")
                    for c in range(DC):
                        nc.tensor.matmul(ps[:], lhsT=wpt[:, o, c, :],
                                         rhs=attn[:, c, nsl],
                                         start=(c == 0), stop=(c == DC - 1))
                    nc.vector.tensor_add(h_sb[:, o, :], ps[:], xo[:])
                    yield
                if skip_mlp:
                    for o in range(DC):
                        nc.sync.dma_start(
                            yo[:].rearrange("c p t -> p c t")[:, o, nsl],
                            h_sb[:, o, :])
                        yield
                    return
                # LN2 (stats on bf16 copies; normalize = scale-only, folded
                # mean via centered w1)
                def src_sq2(c):
                    hb = spool.tile([P, TT], BF, tag="hb")
                    nc.vector.tensor_copy(hb[:], h_sb[:, c, :])
                    sq = spool.tile([P, TT], BF, tag="sq")
                    nc.vector.tensor_mul(sq[:], hb[:], hb[:])
                    return hb[:], sq[:]
                st = stats_pair(src_sq2, 0)
                yield
                bb = finish_stats(st, gpool, cpool, spool)
                for c in range(DC):
                    nc.vector.tensor_mul(xh2[:, c, :], h_sb[:, c, :], bb)
                yield
                # grouped MLP: fc1/silu for 8 hidden chunks, then partial fc2
                # accumulated into h_sb (residual base) in-place
                for grp in range(NG):
                    g = gpool.tile([P, HG, TT], BF, tag="g")
                    for k in range(HG):
                        hc = grp * HG + k
                        w1t = fpool.tile([P, DC, P], BF, tag="w1")
                        nc.sync.dma_start(w1t[:], w1[hc, :, :, :])
                        ps = pp1.tile([P, TT], F32, tag="ta" if k % 2 == 0 else "tb")
                        for c in range(DC):
                            nc.tensor.matmul(ps[:], lhsT=w1t[:, c, :],
                                             rhs=xh2[:, c, :],
                                             start=(c == 0), stop=(c == DC - 1))
                        nc.scalar.activation(g[:, k, :], ps[:], AF.Silu)
                        yield
                    for o in range(DC):
                        w2t = w2pool.tile([P, HG, P], BF, tag="w2")
                        nc.sync.dma_start(w2t[:], w2[o, :, grp, :, :])
                        ps = pp1.tile([P, TT], F32, tag="ta" if o % 2 == 0 else "tb")
                        for k in range(HG):
                            nc.tensor.matmul(ps[:], lhsT=w2t[:, k, :],
                                             rhs=g[:, k, :],
                                             start=(k == 0), stop=(k == HG - 1))
                        nc.vector.tensor_add(h_sb[:, o, :], h_sb[:, o, :], ps[:])
                        yield
                for o in range(DC):
                    nc.sync.dma_start(
                        yo[:].rearrange("c p t -> p c t")[:, o, nsl],
                        h_sb[:, o, :])
                yield

            def drain(gen):
                if gen is not None:
                    for _ in gen:
                        pass

            nc.sync.dma_start(wpt, wp[:].rearrange("o p c m -> p o c m"))

            if skip_attn:
                nc.vector.memset(attn[:], 0.01)
                for hp in range(DC):
                    drain(emit_kq(hp))
                drain(tail(0))
                drain(tail(1))
                return

            # n=0: interleave attention with K/Q of the next head-pair.
            # n=1: interleave attention with tail(0) so PE chews proj/MLP of
            # sequence-half 0 while ACT runs the exp wave of half 1.
            drain(emit_kq(0))
            for hp in range(DC):
                kq_next = emit_kq(hp + 1) if hp + 1 < DC else None
                for step, _ in enumerate(emit_attn(hp, 0)):
                    if kq_next is not None and step % 2 == 1:
                        next(kq_next, None)
                        next(kq_next, None)
                drain(kq_next)
            tail0 = tail(0)
            for hp in range(DC):
                for _ in emit_attn(hp, 1):
                    next(tail0, None)
            drain(tail0)
            drain(tail(1))


_CACHE = {}


def _get_nc():
    if "nc" not in _CACHE:
        _CACHE["nc"] = build_nc()
    return _CACHE["nc"]


def _prep_weights(ln1_w, ln1_b, qkv_w, qkv_b, proj_w, proj_b,
                  ln2_w, ln2_b, fc1_w, fc1_b, fc2_w, fc2_b):
    """Fold LN affine params AND the LN mean-subtraction into the adjacent
    weights (column centering; exact algebra), lay out as [K-chunk, K-in-chunk,
    M] bf16."""
    qkv_w = np.asarray(qkv_w, np.float32)
    fold1 = np.asarray(ln1_w, np.float32)[:, None] * qkv_w
    bias1 = np.asarray(qkv_b, np.float32) + np.asarray(ln1_b, np.float32) @ qkv_w
    fold1 = fold1 - fold1.mean(axis=0, keepdims=True)   # center: folds mean-sub
    fc1 = np.asarray(fc1_w, np.float32)
    fold2 = np.asarray(ln2_w, np.float32)[:, None] * fc1
    bias2 = np.asarray(fc1_b, np.float32) + np.asarray(ln2_b, np.float32) @ fc1
    fold2 = fold2 - fold2.mean(axis=0, keepdims=True)
    for name, b in (("qkv", bias1), ("proj", np.asarray(proj_b, np.float32)),
                    ("fc1", bias2), ("fc2", np.asarray(fc2_b, np.float32))):
        assert not np.any(b), (
            f"{name} effective bias is nonzero; bias support not emitted in this kernel")

    def chunk4(a, n_out, m_out):
        # [D_in, D_out] -> [out-chunk, p, in-chunk, m] with contiguous per-p tile
        return np.ascontiguousarray(
            a.reshape(a.shape[0] // P, P, n_out, m_out).transpose(2, 1, 0, 3)
        ).astype(BF_NP)

    wq_ = chunk4(fold1[:, 0:D], DC, P)
    wk_ = chunk4(fold1[:, D:2 * D], DC, P)
    wv_ = chunk4(fold1[:, 2 * D:3 * D], 2, 512)
    wp_ = chunk4(np.asarray(proj_w, np.float32), DC, P)
    w1_ = chunk4(fold2, HC, P)
    w2_ = chunk4(np.asarray(fc2_w, np.float32), DC, P)
    # fc2 grouped layout: [DC, P, HC, P] -> [DC, P, NG, HG, P]
    w2_ = np.ascontiguousarray(w2_.reshape(DC, P, NG, HG, P))
    return dict(wq=wq_, wk=wk_, wv=wv_, wp=wp_, w1=w1_, w2=w2_)


def kernel(x, ln1_w, ln1_b, qkv_w, qkv_b, proj_w, proj_b,
           ln2_w, ln2_b, fc1_w, fc1_b, fc2_w, fc2_b):
    x = np.asarray(x, np.float32)
    B = x.shape[0]
    assert x.shape == (B, T, D) and B * 2 == 8, f"unexpected x shape {x.shape}"
    weights = _prep_weights(ln1_w, ln1_b, qkv_w, qkv_b, proj_w, proj_b,
                            ln2_w, ln2_b, fc1_w, fc1_b, fc2_w, fc2_b)
    nc = _get_nc()

    in_maps = []
    for c in range(8):
        b, s = c // 2, c % 2
        if s == 0:
            xp = x[b]
        else:
            xp = np.concatenate([x[b, TOWN:], x[b, :TOWN]], axis=0)
        xb_ = np.ascontiguousarray(xp.T).reshape(DC, P, T).astype(BF_NP)
        xr_ = np.ascontiguousarray(xp[:TOWN].T).reshape(DC, P, TOWN)
        in_maps.append({"xb": xb_, "xr": xr_, **weights})

    res = run_bass_kernel_spmd(nc, in_maps, core_ids=list(range(8)))

    y = np.empty((B, T, D), np.float32)
    for c in range(8):
        b, s = c // 2, c % 2
        yc = res.results[c]["y"].reshape(D, TOWN)  # [dim, own tokens]
        y[b, s * TOWN:(s + 1) * TOWN, :] = yc.T
    return y
